# revision 47
# baseline (speedup 1.0000x reference)
"""Trainium2 Bass kernel for nn_CausalSelfAttention_17368847745133.

Sharding (8 NeuronCores): core c owns heads (2c, 2c+1) for ALL 4 batches
(Megatron column-parallel c_attn / row-parallel c_proj over heads).  The
program is specialized at build time on Kb[b] = ceil(l_b/128): query/key
projections, S = qk^T, exp, and PV are emitted only for the live region
q,k < 128*Kb[b]; rows q >= l_b of the output are the uniform-softmax mean
of v (the reference's -1e8 mask makes those rows exactly uniform), blended
in with host-provided 0/1 masks, so results stay correct for the given l.

Per core / batch b (all matmuls bf16, statistics fp32):
  V^T  [128,2048] : matmul(lhsT=Wv slice, rhs=xT)    then one
                    dma_start_transpose per head -> V [kpos,2,65] tiles
                    (col 64 = ones for the softmax denominator row)
  ypad [128,1]    : DVE row-sum of V^T / T   (mean of v over ALL k)
  qT/kT[128,LQ]   : matmul(lhsT=Wq|k slice, rhs=xT), trimmed to LQ
  S^T  [k,q]      : matmul(lhsT=kT head, rhs=qT head), head pair packed
                    on PE row groups 0-63 / 64-127, block-causal trimmed
  P^T  = exp(S^T/8) * m01 on diagonal blocks
  U    [65,q]     : matmul(lhsT=[V|1], rhs=P^T); row 64 = denominator
  y^T  = U[0:64]*bcast(qm/den) + ypad*(1-qm);  rows q >= 128*Kb = ypad
  o^T  partial [1024,2048] = matmul(lhsT=Wp rows, rhs=y^T); host sums the
                    8 per-core partials, transposes, adds b_proj.

Batch b+1's V^T and q/k projections are interleaved into batch b's
attention j-loop to keep the PE busy while the ACT engine runs exp.
"""

import math

import ml_dtypes
import numpy as np

import concourse.bass as bass
import concourse.mybir as mybir
import concourse.tile as tile
from concourse import bacc
from concourse.bass_utils import run_bass_kernel_spmd

P = 128
B, T, C = 4, 2048, 1024
H, D = 16, 64
F32 = mybir.dt.float32
BF16 = mybir.dt.bfloat16
SCALE = 0.125    # 1/sqrt(64)

_CACHED = {}
_POPS = True


def build_nc(kb, debug=False):
    """kb: tuple of 4 ints, Kb[b] = ceil(clamp(l_b,1,T)/128) in 1..16."""
    nc = bacc.Bacc(trn_type="TRN2", target_bir_lowering=False)

    xT = nc.dram_tensor("xT", [B, C, T], BF16, kind="ExternalInput")
    wq = nc.dram_tensor("wq", [P, 8, P], BF16, kind="ExternalInput")
    wk = nc.dram_tensor("wk", [P, 8, P], BF16, kind="ExternalInput")
    wv = nc.dram_tensor("wv", [P, 8, P], BF16, kind="ExternalInput")
    wp = nc.dram_tensor("wp", [P, C], BF16, kind="ExternalInput")
    ident = nc.dram_tensor("ident", [P, D], BF16, kind="ExternalInput")
    m01 = nc.dram_tensor("m01", [P, P], BF16, kind="ExternalInput")
    qmA = nc.dram_tensor("qmA", [P, 512], F32, kind="ExternalInput")
    qmnt = nc.dram_tensor("qmnt", [P, B, 512], F32, kind="ExternalInput")
    oT = nc.dram_tensor("oT", [B, C, T], BF16, kind="ExternalOutput")
    if debug:
        d_q = nc.dram_tensor("d_q", [P, B, T], BF16, kind="ExternalOutput")
        d_k = nc.dram_tensor("d_k", [P, B, T], BF16, kind="ExternalOutput")
        d_V = nc.dram_tensor("d_V", [P, B, 16, 2, D + 1], BF16,
                             kind="ExternalOutput")
        d_y = nc.dram_tensor("d_y", [P, B, T], BF16, kind="ExternalOutput")
        d_yp = nc.dram_tensor("d_yp", [P, B], F32, kind="ExternalOutput")

    LQ = [128 * k for k in kb]
    JB = [(k + 3) // 4 for k in kb]
    QE = [LQ[b] - 512 * (JB[b] - 1) for b in range(B)]

    with tile.TileContext(nc) as tc:
        with tc.tile_pool(name="misc", bufs=1) as misc, \
             tc.tile_pool(name="xp", bufs=2) as xpool, \
             tc.tile_pool(name="vtp", bufs=2) as vtpool, \
             tc.tile_pool(name="qk", bufs=1) as qkpool, \
             tc.tile_pool(name="vp", bufs=1) as vpool, \
             tc.tile_pool(name="yp", bufs=1) as ypool, \
             tc.tile_pool(name="pt", bufs=4) as ptpool, \
             tc.tile_pool(name="nrm", bufs=3) as nrm, \
             tc.tile_pool(name="ob", bufs=3) as obpool, \
             tc.tile_pool(name="rdram", bufs=2, space="DRAM") as rdram, \
             tc.tile_pool(name="psS", bufs=2, space="PSUM") as psS, \
             tc.tile_pool(name="psU", bufs=2, space="PSUM") as psU, \
             tc.tile_pool(name="psV", bufs=2, space="PSUM") as psV:

            # ---- constants ----
            m01_sb = misc.tile([P, P], BF16, tag="m01")
            qmA_sb = misc.tile([P, 512], F32, tag="qmA")
            qmnt_sb = misc.tile([P, B, 512], F32, tag="qmnt")
            wq_sb = misc.tile([P, 8, P], BF16, tag="wq")
            wk_sb = misc.tile([P, 8, P], BF16, tag="wk")
            wv_sb = misc.tile([P, 8, P], BF16, tag="wv")
            wp_sb = misc.tile([P, C], BF16, tag="wp")
            id_sb = misc.tile([P, D], BF16, tag="ident")
            for dst, src in ((m01_sb, m01), (qmA_sb, qmA), (qmnt_sb, qmnt),
                             (wq_sb, wq), (wk_sb, wk), (wv_sb, wv),
                             (wp_sb, wp), (id_sb, ident)):
                nc.sync.dma_start(dst, src[:])

            qT = qkpool.tile([P, B, T], BF16, tag="qT")
            kT = qkpool.tile([P, B, T], BF16, tag="kT")
            V_sb = vpool.tile([P, B, 16, 2, D + 1], BF16, tag="V")
            yT = ypool.tile([P, B, T], BF16, tag="yT")
            ypad = misc.tile([P, B], F32, tag="ypad")
            den_sb = misc.tile([P, 512], F32, tag="den")
            nc.vector.memset(den_sb, 1.0)
            for b in range(B):
                nc.vector.memset(V_sb[:, b, :, :, D:D + 1], 1.0)

            xtiles = {}

            def load_x(b):
                xb = xpool.tile([P, 8, T], BF16, tag="x", name=f"x{b}")
                for ct in range(8):
                    nc.sync.dma_start(xb[:, ct], xT[b, ct * P:(ct + 1) * P, :])
                xtiles[b] = xb

            def v_proj_chunk(b, pb):
                """V^T projection for 512 positions; pb in 0..3."""
                xb = xtiles[b]
                vtp = self_vtp[b]
                ps = psV.tile([P, 512], F32, tag="psV", name=f"vp{b}_{pb}")
                for kt in range(8):
                    nc.tensor.matmul(ps, wv_sb[:, kt, :],
                                     xb[:, kt, pb * 512:(pb + 1) * 512],
                                     start=(kt == 0), stop=(kt == 7))
                nc.vector.tensor_copy(vtp[:, pb * 512:(pb + 1) * 512], ps)

            def v_finish(b, h):
                """Transpose head h's V^T into V_sb k-tiles via PE transpose
                (dma_start_transpose is not dependency-tracked by Tile and
                races its consumers on HW)."""
                vtp = self_vtp[b]
                for i in range(4):
                    psv = psV.tile([P, 4, D], BF16, tag="psV",
                                   name=f"tp{b}_{h}_{i}")
                    for t in range(4):
                        nc.tensor.transpose(
                            psv[:, t, :],
                            vtp[h * D:(h + 1) * D,
                                (4 * i + t) * P:(4 * i + t + 1) * P],
                            id_sb[h * D:(h + 1) * D, :])
                    nc.vector.tensor_copy(
                        V_sb[:, b, 4 * i:4 * i + 4, h, 0:D], psv)
                if h == 1:
                    nc.vector.tensor_reduce(
                        ypad[:, b:b + 1], vtp, axis=mybir.AxisListType.X,
                        op=mybir.AluOpType.add)
                    nc.vector.tensor_scalar_mul(ypad[:, b:b + 1],
                                                ypad[:, b:b + 1], 1.0 / T)

            def qk_chunk(b, side, ch):
                """q or k projection for 512 positions; ch in 0..ceil(LQ/512)-1."""
                xb = xtiles[b]
                w_sb = (wq_sb, wk_sb)[side]
                dst = (qT, kT)[side]
                width = min(512, LQ[b] - 512 * ch)
                ps = psV.tile([P, 512], F32, tag="psV", name=f"qk{b}_{side}_{ch}")
                for kt in range(8):
                    nc.tensor.matmul(ps[:, 0:width], w_sb[:, kt, :],
                                     xb[:, kt, 512 * ch:512 * ch + width],
                                     start=(kt == 0), stop=(kt == 7))
                nc.vector.tensor_copy(dst[:, b, 512 * ch:512 * ch + width],
                                      ps[:, 0:width])

            self_vtp = {}

            def prep_batch(b):
                """Emit all projection work for batch b as a flat op list."""
                self_vtp[b] = vtpool.tile([P, T], BF16, tag="vt", name=f"vt{b}")
                nch = (LQ[b] + 511) // 512
                ops = [lambda b=b, pb=pb: v_proj_chunk(b, pb) for pb in range(4)]
                ops += [lambda b=b, h=h: v_finish(b, h) for h in range(2)]
                for ch in range(nch):
                    ops.append(lambda b=b, ch=ch: qk_chunk(b, 0, ch))
                    ops.append(lambda b=b, ch=ch: qk_chunk(b, 1, ch))
                return ops

            def attention(b, bg_ops):
                """Attention for batch b; bg_ops are interleaved (next batch
                projections) to fill PE time while ACT runs exp."""
                Kb = kb[b]
                for j in range(JB[b]):
                    Nj = 512 if j < JB[b] - 1 else QE[b]
                    nkt = min(4 * (j + 1), Kb)
                    blk = slice(512 * j, 512 * j + Nj)
                    Upr = [psU.tile([D + 1, 512], F32, tag="psU",
                                    name=f"U{b}_{j}_{par}") for par in range(2)]

                    def s_exp(kt, j=j, Nj=Nj):
                        dlt = 128 * kt - 512 * j
                        c0 = max(dlt, 0)
                        ss = psS.tile([P, 2, 512], F32, tag="psS")
                        for par in range(2):
                            p0 = par * D
                            nc.tensor.matmul(
                                ss[:, par, c0:Nj],
                                kT[p0:p0 + D, b, kt * P:(kt + 1) * P],
                                qT[p0:p0 + D, b, 512 * j + c0:512 * j + Nj],
                                start=True, stop=True)
                        pt = ptpool.tile([P, 2, 512], BF16, tag="pt")
                        nc.scalar.activation(
                            pt[:, :, c0:Nj], ss[:, :, c0:Nj],
                            mybir.ActivationFunctionType.Exp,
                            bias=0.0, scale=SCALE)
                        ptm = None
                        if dlt >= 0:
                            # masked diagonal tile goes to its own tile so the
                            # PV matmul has a hard dependency on the mask
                            ptm = ptpool.tile([P, 2, P], BF16, tag="ptm")
                            nc.vector.tensor_mul(
                                out=ptm,
                                in0=pt[:, :, c0:c0 + P],
                                in1=m01_sb[:, None, :].to_broadcast([P, 2, P]))
                        return pt, ptm, c0

                    def pv(kt, pt, ptm, c0, Nj=Nj, nkt=nkt):
                        # columns < c0 are causally dead for this k tile:
                        # accumulate only the live range (kt==0 has c0==0,
                        # so every column is initialized by the first step)
                        for par in range(2):
                            if ptm is None:
                                nc.tensor.matmul(
                                    Upr[par][:, c0:Nj],
                                    V_sb[:, b, kt, par, :],
                                    pt[:, par, c0:Nj],
                                    start=(kt == 0), stop=(kt == nkt - 1),
                                    skip_group_check=True)
                            else:
                                nc.tensor.matmul(
                                    Upr[par][:, c0:c0 + P],
                                    V_sb[:, b, kt, par, :],
                                    ptm[:, par, :],
                                    start=(kt == 0),
                                    stop=(kt == nkt - 1 and c0 + P >= Nj),
                                    skip_group_check=True)
                                if c0 + P < Nj:
                                    nc.tensor.matmul(
                                        Upr[par][:, c0 + P:Nj],
                                        V_sb[:, b, kt, par, :],
                                        pt[:, par, c0 + P:Nj],
                                        start=(kt == 0), stop=(kt == nkt - 1),
                                        skip_group_check=True)

                    prev = None
                    for kt in range(nkt):
                        cur = s_exp(kt)
                        if prev is not None:
                            pv(kt - 1, *prev)
                        prev = cur
                        if bg_ops and j > 0 and _POPS:
                            bg_ops.pop(0)()
                    pv(nkt - 1, *prev)

                    # per-j epilogue: stash denom row + unnormalized y
                    for par in range(2):
                        U = Upr[par]
                        r = b * 32 + par * 4 + j
                        dtf = nrm.tile([D + 1, 512], F32, tag="dt")
                        if par == 0:
                            nc.scalar.copy(dtf[D:D + 1, 0:Nj], U[D:D + 1, 0:Nj])
                        else:
                            nc.vector.tensor_copy(dtf[D:D + 1, 0:Nj],
                                                  U[D:D + 1, 0:Nj])
                        nc.gpsimd.dma_start(den_sb[r:r + 1, 0:Nj],
                                            dtf[D:D + 1, 0:Nj])
                        if par == 0:
                            nc.vector.tensor_copy(yT[0:D, b, blk], U[0:D, 0:Nj])
                        else:
                            yt2 = nrm.tile([D, 512], BF16, tag="ytmp")
                            nc.vector.tensor_copy(yt2[:, 0:Nj], U[0:D, 0:Nj])
                            nc.gpsimd.dma_start(yT[D:P, b, blk], yt2[:, 0:Nj])
                while bg_ops:
                    bg_ops.pop(0)()

            def norm_batch(b):
                r0 = b * 32
                dq = den_sb[r0:r0 + 8, :]
                nc.vector.reciprocal(dq, dq)
                nc.vector.tensor_mul(out=dq, in0=dq, in1=qmA_sb[r0:r0 + 8, :])
                dend = rdram.tile([8, 512], F32, tag="dend", name=f"dend{b}")
                nc.sync.dma_start(dend, dq)
                for j in range(JB[b]):
                    Nj = 512 if j < JB[b] - 1 else QE[b]
                    blk = slice(512 * j, 512 * j + Nj)
                    rb = nrm.tile([P, 512], F32, tag="rb")
                    for par in range(2):
                        row = dend[par * 4 + j:par * 4 + j + 1, 0:Nj]
                        src = bass.AP(tensor=row.tensor, offset=row.offset,
                                      ap=[[0, D]] + list(row.ap[1:]))
                        nc.sync.dma_start(rb[par * D:(par + 1) * D, 0:Nj], src)
                    ys = yT[:, b, blk]
                    nc.vector.tensor_mul(out=ys, in0=ys, in1=rb[:, 0:Nj])
                    if j == JB[b] - 1:
                        t3 = nrm.tile([P, 512], F32, tag="t3")
                        nc.vector.tensor_mul(
                            out=t3[:, 0:Nj], in0=qmnt_sb[:, b, 0:Nj],
                            in1=ypad[:, b:b + 1].to_broadcast([P, Nj]))
                        nc.vector.tensor_add(out=ys, in0=ys, in1=t3[:, 0:Nj])
                if LQ[b] < T:
                    nc.vector.tensor_copy(
                        yT[:, b, LQ[b]:T],
                        ypad[:, b:b + 1].to_broadcast([P, T - LQ[b]]))

            def out_chunk(b, mt, qb):
                ps = psV.tile([P, 512], F32, tag="psV", name=f"o{b}_{mt}_{qb}")
                nc.tensor.matmul(ps, wp_sb[:, mt * P:(mt + 1) * P],
                                 yT[:, b, qb * 512:(qb + 1) * 512],
                                 start=True, stop=True)
                ob = obpool.tile([P, 512], BF16, tag="ob")
                if (mt + qb) % 2 == 0:
                    nc.vector.tensor_copy(ob, ps)
                else:
                    nc.scalar.copy(ob, ps)
                nc.gpsimd.dma_start(
                    oT[b, mt * P:(mt + 1) * P,
                       qb * 512:(qb + 1) * 512], ob)

            def out_ops(b):
                return [lambda b=b, mt=mt, qb=qb: out_chunk(b, mt, qb)
                        for mt in range(8) for qb in range(4)]

            # ---- schedule ----
            load_x(0)
            for op in prep_batch(0):
                op()
            for b in range(B):
                bg = out_ops(b - 1) if b > 0 else []
                if b + 1 < B:
                    load_x(b + 1)
                    bg += prep_batch(b + 1)
                attention(b, bg)
                norm_batch(b)
            for op in out_ops(B - 1):
                op()

            if debug:
                for b in range(B):
                    nc.gpsimd.dma_start(d_q[:, b, 0:LQ[b]], qT[:, b, 0:LQ[b]])
                    nc.gpsimd.dma_start(d_k[:, b, 0:LQ[b]], kT[:, b, 0:LQ[b]])
                nc.gpsimd.dma_start(d_V[:], V_sb)
                nc.gpsimd.dma_start(d_y[:], yT)
                nc.gpsimd.dma_start(d_yp[:], ypad)

    nc.compile()
    return nc


def _bf(a):
    return np.ascontiguousarray(np.asarray(a)).astype(ml_dtypes.bfloat16)


def _prep_inputs(x, l, W_attn, W_proj):
    x = np.asarray(x, dtype=np.float32)
    W_attn = np.asarray(W_attn, dtype=np.float32)
    W_proj = np.asarray(W_proj, dtype=np.float32)
    lv = np.clip(np.asarray(l).astype(np.int64), 1, T)
    kbs = [int(math.ceil(int(lv[b]) / 128.0)) for b in range(B)]

    xTb = np.stack([np.ascontiguousarray(x[b].T) for b in range(B)])
    xTb = xTb.astype(ml_dtypes.bfloat16)

    m01 = np.where(np.arange(P)[:, None] > np.arange(P)[None, :],
                   0.0, 1.0).astype(ml_dtypes.bfloat16)

    qmA = np.zeros((P, 512), dtype=np.float32)
    qmnt = np.zeros((P, B, 512), dtype=np.float32)
    for b in range(B):
        lb = int(lv[b])
        jb = (kbs[b] + 3) // 4
        qrow = (np.arange(T) < lb).astype(np.float32)
        for j in range(jb):
            seg = qrow[512 * j:512 * (j + 1)]
            for par in range(2):
                qmA[b * 32 + par * 4 + j, :] = seg
        tail = qrow[512 * (jb - 1):512 * jb]
        qmnt[:, b, 0:len(tail)] = 1.0 - tail[None, :]

    in_maps = []
    for c in range(8):
        cs = slice(c * P, (c + 1) * P)
        wqc = _bf(W_attn[:, 0 * C:1 * C][:, cs].reshape(8, P, P).transpose(1, 0, 2))
        wkc = _bf(W_attn[:, 1 * C:2 * C][:, cs].reshape(8, P, P).transpose(1, 0, 2))
        wvc = _bf(W_attn[:, 2 * C:3 * C][:, cs].reshape(8, P, P).transpose(1, 0, 2))
        wpc = _bf(W_proj[cs, :])
        in_maps.append({
            "xT": xTb, "wq": wqc, "wk": wkc, "wv": wvc, "wp": wpc,
            "ident": np.tile(np.eye(D, dtype=np.float32),
                             (2, 1)).astype(ml_dtypes.bfloat16),
            "m01": m01, "qmA": qmA, "qmnt": qmnt,
        })
    return in_maps, tuple(kbs)


def kernel(x, l, W_attn, b_attn, W_proj, b_proj, _want_profile=False):
    b_attn = np.asarray(b_attn, dtype=np.float32)
    b_proj = np.asarray(b_proj, dtype=np.float32)
    assert not np.any(b_attn), "nonzero b_attn not supported by this kernel"

    in_maps, kbs = _prep_inputs(x, l, W_attn, W_proj)
    if kbs not in _CACHED:
        _CACHED[kbs] = build_nc(kbs)
    nc = _CACHED[kbs]

    res = run_bass_kernel_spmd(nc, in_maps, core_ids=list(range(8)),
                               trace=_want_profile)

    acc = np.zeros((B, C, T), dtype=np.float32)
    for c in range(8):
        acc += res.results[c]["oT"].astype(np.float32)
    out = np.ascontiguousarray(acc.transpose(0, 2, 1)) + b_proj[None, None, :]
    if _want_profile:
        return out, res
    return out


# revision 57
# speedup vs baseline: 1.0726x; 1.0726x over previous
"""Trainium2 Bass kernel for nn_CausalSelfAttention_17368847745133.

Sharding (8 NeuronCores): core c owns heads (2c, 2c+1) for ALL 4 batches
(Megatron column-parallel c_attn / row-parallel c_proj over heads).  The
program is specialized at build time on Kb[b] = ceil(l_b/128): query/key
projections, S = qk^T, exp, and PV are emitted only for the live region
q,k < 128*Kb[b]; rows q >= l_b of the output are the uniform-softmax mean
of v (the reference's -1e8 mask makes those rows exactly uniform), blended
in with host-provided 0/1 masks, so results stay correct for the given l.

Per core / batch b (all matmuls bf16, statistics fp32):
  V^T  [128,2048] : matmul(lhsT=Wv slice, rhs=xT)    then one
                    dma_start_transpose per head -> V [kpos,2,65] tiles
                    (col 64 = ones for the softmax denominator row)
  ypad [128,1]    : DVE row-sum of V^T / T   (mean of v over ALL k)
  qT/kT[128,LQ]   : matmul(lhsT=Wq|k slice, rhs=xT), trimmed to LQ
  S^T  [k,q]      : matmul(lhsT=kT head, rhs=qT head), head pair packed
                    on PE row groups 0-63 / 64-127, block-causal trimmed
  P^T  = exp(S^T/8) * m01 on diagonal blocks
  U    [65,q]     : matmul(lhsT=[V|1], rhs=P^T); row 64 = denominator
  y^T  = U[0:64]*bcast(qm/den) + ypad*(1-qm);  rows q >= 128*Kb = ypad
  o^T  partial [1024,2048] = matmul(lhsT=Wp rows, rhs=y^T); host sums the
                    8 per-core partials, transposes, adds b_proj.

Batch b+1's V^T and q/k projections are interleaved into batch b's
attention j-loop to keep the PE busy while the ACT engine runs exp.
"""

import math

import ml_dtypes
import numpy as np

import concourse.bass as bass
import concourse.mybir as mybir
import concourse.tile as tile
from concourse import bacc
from concourse.bass_utils import run_bass_kernel_spmd

P = 128
B, T, C = 4, 2048, 1024
H, D = 16, 64
F32 = mybir.dt.float32
BF16 = mybir.dt.bfloat16
SCALE = 0.125    # 1/sqrt(64)

_CACHED = {}
_POPS = True


def build_nc(kb, debug=False):
    """kb: tuple of 4 ints, Kb[b] = ceil(clamp(l_b,1,T)/128) in 1..16."""
    nc = bacc.Bacc(trn_type="TRN2", target_bir_lowering=False)

    xT = nc.dram_tensor("xT", [B, C, T], BF16, kind="ExternalInput")
    wq = nc.dram_tensor("wq", [P, 8, P], BF16, kind="ExternalInput")
    wk = nc.dram_tensor("wk", [P, 8, P], BF16, kind="ExternalInput")
    wv = nc.dram_tensor("wv", [P, 8, P], BF16, kind="ExternalInput")
    wp = nc.dram_tensor("wp", [P, C], BF16, kind="ExternalInput")
    ident = nc.dram_tensor("ident", [P, D], BF16, kind="ExternalInput")
    m01 = nc.dram_tensor("m01", [P, P], BF16, kind="ExternalInput")
    qmA = nc.dram_tensor("qmA", [P, 512], F32, kind="ExternalInput")
    qmnt = nc.dram_tensor("qmnt", [P, B, 512], F32, kind="ExternalInput")
    oT = nc.dram_tensor("oT", [B, C, T], BF16, kind="ExternalOutput")
    if debug:
        d_q = nc.dram_tensor("d_q", [P, B, T], BF16, kind="ExternalOutput")
        d_k = nc.dram_tensor("d_k", [P, B, T], BF16, kind="ExternalOutput")
        d_V = nc.dram_tensor("d_V", [P, B, 16, 2, D + 1], BF16,
                             kind="ExternalOutput")
        d_y = nc.dram_tensor("d_y", [P, B, T], BF16, kind="ExternalOutput")
        d_yp = nc.dram_tensor("d_yp", [P, B], F32, kind="ExternalOutput")

    LQ = [128 * k for k in kb]
    JB = [(k + 3) // 4 for k in kb]
    QE = [LQ[b] - 512 * (JB[b] - 1) for b in range(B)]

    with tile.TileContext(nc) as tc:
        with tc.tile_pool(name="misc", bufs=1) as misc, \
             tc.tile_pool(name="xp", bufs=2) as xpool, \
             tc.tile_pool(name="vtp", bufs=2) as vtpool, \
             tc.tile_pool(name="qk", bufs=1) as qkpool, \
             tc.tile_pool(name="vp", bufs=1) as vpool, \
             tc.tile_pool(name="yp", bufs=1) as ypool, \
             tc.tile_pool(name="pt", bufs=4) as ptpool, \
             tc.tile_pool(name="nrm", bufs=3) as nrm, \
             tc.tile_pool(name="ob", bufs=3) as obpool, \
             tc.tile_pool(name="rdram", bufs=2, space="DRAM") as rdram, \
             tc.tile_pool(name="psS", bufs=3, space="PSUM") as psS, \
             tc.tile_pool(name="psU", bufs=2, space="PSUM") as psU:

            # ---- constants ----
            m01_sb = misc.tile([P, P], BF16, tag="m01")
            qmA_sb = misc.tile([P, 512], F32, tag="qmA")
            qmnt_sb = misc.tile([P, B, 512], F32, tag="qmnt")
            wq_sb = misc.tile([P, 8, P], BF16, tag="wq")
            wk_sb = misc.tile([P, 8, P], BF16, tag="wk")
            wv_sb = misc.tile([P, 8, P], BF16, tag="wv")
            wp_sb = misc.tile([P, C], BF16, tag="wp")
            id_sb = misc.tile([P, D], BF16, tag="ident")
            for dst, src in ((m01_sb, m01), (qmA_sb, qmA), (qmnt_sb, qmnt),
                             (wq_sb, wq), (wk_sb, wk), (wv_sb, wv),
                             (wp_sb, wp), (id_sb, ident)):
                nc.sync.dma_start(dst, src[:])

            qT = qkpool.tile([P, B, T], BF16, tag="qT")
            kT = qkpool.tile([P, B, T], BF16, tag="kT")
            V_sb = vpool.tile([P, B, 16, 2, D + 1], BF16, tag="V")
            yT = ypool.tile([P, B, T], BF16, tag="yT")
            ypad = misc.tile([P, B], F32, tag="ypad")
            den_sb = misc.tile([P, 512], F32, tag="den")
            nc.vector.memset(den_sb, 1.0)
            for b in range(B):
                nc.vector.memset(V_sb[:, b, :, :, D:D + 1], 1.0)

            xtiles = {}

            def load_x(b):
                xb = xpool.tile([P, 8, T], BF16, tag="x", name=f"x{b}")
                for ct in range(8):
                    nc.sync.dma_start(xb[:, ct], xT[b, ct * P:(ct + 1) * P, :])
                xtiles[b] = xb

            def v_proj_chunk(b, pb):
                """V^T projection for 512 positions (pb in 0..3), transposed
                into V_sb k-tiles 4*pb..4*pb+3 via PE transpose
                (dma_start_transpose is not dependency-tracked by Tile and
                races its consumers on HW)."""
                xb = xtiles[b]
                vtp = self_vtp[b]
                ps = psS.tile([P, 512], F32, tag="psS", name=f"vp{b}_{pb}")
                for kt in range(8):
                    nc.tensor.matmul(ps, wv_sb[:, kt, :],
                                     xb[:, kt, pb * 512:(pb + 1) * 512],
                                     start=(kt == 0), stop=(kt == 7))
                nc.vector.tensor_copy(vtp[:, pb * 512:(pb + 1) * 512], ps)
                for h in range(2):
                    psv = psS.tile([P, 4, D], BF16, tag="psS",
                                   name=f"tp{b}_{pb}_{h}")
                    for t in range(4):
                        nc.tensor.transpose(
                            psv[:, t, :],
                            vtp[h * D:(h + 1) * D,
                                (4 * pb + t) * P:(4 * pb + t + 1) * P],
                            id_sb[h * D:(h + 1) * D, :])
                    nc.vector.tensor_copy(
                        V_sb[:, b, 4 * pb:4 * pb + 4, h, 0:D], psv)

            def ypad_reduce(b):
                vtp = self_vtp[b]
                nc.vector.tensor_reduce(
                    ypad[:, b:b + 1], vtp, axis=mybir.AxisListType.X,
                    op=mybir.AluOpType.add)
                nc.vector.tensor_scalar_mul(ypad[:, b:b + 1],
                                            ypad[:, b:b + 1], 1.0 / T)

            def qk_chunk(b, side, ch):
                """q or k projection for 512 positions; ch in 0..ceil(LQ/512)-1."""
                xb = xtiles[b]
                w_sb = (wq_sb, wk_sb)[side]
                dst = (qT, kT)[side]
                width = min(512, LQ[b] - 512 * ch)
                ps = psS.tile([P, 512], F32, tag="psS", name=f"qk{b}_{side}_{ch}")
                for kt in range(8):
                    nc.tensor.matmul(ps[:, 0:width], w_sb[:, kt, :],
                                     xb[:, kt, 512 * ch:512 * ch + width],
                                     start=(kt == 0), stop=(kt == 7))
                nc.vector.tensor_copy(dst[:, b, 512 * ch:512 * ch + width],
                                      ps[:, 0:width])

            self_vtp = {}

            def prep_batch(b):
                """Emit all projection work for batch b as a flat op list,
                ordered so the earliest attention blocks' needs come first."""
                self_vtp[b] = vtpool.tile([P, T], BF16, tag="vt", name=f"vt{b}")
                nch = (LQ[b] + 511) // 512
                ops = []
                for i in range(4):
                    ops.append(lambda b=b, pb=i: v_proj_chunk(b, pb))
                    if i < nch:
                        ops.append(lambda b=b, ch=i: qk_chunk(b, 0, ch))
                        ops.append(lambda b=b, ch=i: qk_chunk(b, 1, ch))
                ops.append(lambda b=b: ypad_reduce(b))
                return ops

            def attention(b, bg_ops, need=None):
                """Attention for batch b; bg_ops are interleaved (next batch
                projections) to fill PE time while ACT runs exp.  need(j)
                gives the minimum number of bg_ops that must be emitted
                before block j (self-projection ordering for the first
                batch)."""
                Kb = kb[b]
                popped = [0]

                def pop():
                    bg_ops.pop(0)()
                    popped[0] += 1

                for j in range(JB[b]):
                    if need is not None:
                        while bg_ops and popped[0] < need(j):
                            pop()
                    Nj = 512 if j < JB[b] - 1 else QE[b]
                    nkt = min(4 * (j + 1), Kb)
                    blk = slice(512 * j, 512 * j + Nj)
                    Upr = [psU.tile([D + 1, 512], F32, tag="psU",
                                    name=f"U{b}_{j}_{par}") for par in range(2)]

                    def s_exp(kt, j=j, Nj=Nj):
                        dlt = 128 * kt - 512 * j
                        c0 = max(dlt, 0)
                        ss = psS.tile([P, 2, 512], F32, tag="psS")
                        for par in range(2):
                            p0 = par * D
                            nc.tensor.matmul(
                                ss[:, par, c0:Nj],
                                kT[p0:p0 + D, b, kt * P:(kt + 1) * P],
                                qT[p0:p0 + D, b, 512 * j + c0:512 * j + Nj],
                                start=True, stop=True)
                        pt = ptpool.tile([P, 2, 512], BF16, tag="pt")
                        nc.scalar.activation(
                            pt[:, :, c0:Nj], ss[:, :, c0:Nj],
                            mybir.ActivationFunctionType.Exp,
                            bias=0.0, scale=SCALE)
                        ptm = None
                        if dlt >= 0:
                            # masked diagonal tile goes to its own tile so the
                            # PV matmul has a hard dependency on the mask
                            ptm = ptpool.tile([P, 2, P], BF16, tag="ptm")
                            nc.vector.tensor_mul(
                                out=ptm,
                                in0=pt[:, :, c0:c0 + P],
                                in1=m01_sb[:, None, :].to_broadcast([P, 2, P]))
                        return pt, ptm, c0

                    def pv(kt, pt, ptm, c0, Nj=Nj, nkt=nkt):
                        # columns < c0 are causally dead for this k tile:
                        # accumulate only the live range (kt==0 has c0==0,
                        # so every column is initialized by the first step)
                        for par in range(2):
                            if ptm is None:
                                nc.tensor.matmul(
                                    Upr[par][:, c0:Nj],
                                    V_sb[:, b, kt, par, :],
                                    pt[:, par, c0:Nj],
                                    start=(kt == 0), stop=(kt == nkt - 1),
                                    skip_group_check=True)
                            else:
                                nc.tensor.matmul(
                                    Upr[par][:, c0:c0 + P],
                                    V_sb[:, b, kt, par, :],
                                    ptm[:, par, :],
                                    start=(kt == 0),
                                    stop=(kt == nkt - 1 and c0 + P >= Nj),
                                    skip_group_check=True)
                                if c0 + P < Nj:
                                    nc.tensor.matmul(
                                        Upr[par][:, c0 + P:Nj],
                                        V_sb[:, b, kt, par, :],
                                        pt[:, par, c0 + P:Nj],
                                        start=(kt == 0), stop=(kt == nkt - 1),
                                        skip_group_check=True)

                    prev = None
                    for kt in range(nkt):
                        cur = s_exp(kt)
                        if prev is not None:
                            pv(kt - 1, *prev)
                        prev = cur
                        if bg_ops and j > 0 and _POPS:
                            pop()
                    pv(nkt - 1, *prev)

                    # per-j epilogue: stash denom row + unnormalized y
                    for par in range(2):
                        U = Upr[par]
                        r = b * 32 + par * 4 + j
                        dtf = nrm.tile([D + 1, 512], F32, tag="dt")
                        if par == 0:
                            nc.scalar.copy(dtf[D:D + 1, 0:Nj], U[D:D + 1, 0:Nj])
                        else:
                            nc.vector.tensor_copy(dtf[D:D + 1, 0:Nj],
                                                  U[D:D + 1, 0:Nj])
                        nc.gpsimd.dma_start(den_sb[r:r + 1, 0:Nj],
                                            dtf[D:D + 1, 0:Nj])
                        if par == 0:
                            nc.vector.tensor_copy(yT[0:D, b, blk], U[0:D, 0:Nj])
                        else:
                            yt2 = nrm.tile([D, 512], BF16, tag="ytmp")
                            nc.vector.tensor_copy(yt2[:, 0:Nj], U[0:D, 0:Nj])
                            nc.gpsimd.dma_start(yT[D:P, b, blk], yt2[:, 0:Nj])
                while bg_ops:
                    bg_ops.pop(0)()

            def norm_batch(b):
                r0 = b * 32
                dq = den_sb[r0:r0 + 8, :]
                nc.vector.reciprocal(dq, dq)
                nc.vector.tensor_mul(out=dq, in0=dq, in1=qmA_sb[r0:r0 + 8, :])
                dend = rdram.tile([8, 512], F32, tag="dend", name=f"dend{b}")
                nc.sync.dma_start(dend, dq)
                for j in range(JB[b]):
                    Nj = 512 if j < JB[b] - 1 else QE[b]
                    blk = slice(512 * j, 512 * j + Nj)
                    rb = nrm.tile([P, 512], F32, tag="rb")
                    for par in range(2):
                        row = dend[par * 4 + j:par * 4 + j + 1, 0:Nj]
                        src = bass.AP(tensor=row.tensor, offset=row.offset,
                                      ap=[[0, D]] + list(row.ap[1:]))
                        nc.sync.dma_start(rb[par * D:(par + 1) * D, 0:Nj], src)
                    ys = yT[:, b, blk]
                    nc.vector.tensor_mul(out=ys, in0=ys, in1=rb[:, 0:Nj])
                    if j == JB[b] - 1:
                        t3 = nrm.tile([P, 512], F32, tag="t3")
                        nc.vector.tensor_mul(
                            out=t3[:, 0:Nj], in0=qmnt_sb[:, b, 0:Nj],
                            in1=ypad[:, b:b + 1].to_broadcast([P, Nj]))
                        nc.vector.tensor_add(out=ys, in0=ys, in1=t3[:, 0:Nj])
                if LQ[b] < T:
                    nc.vector.tensor_copy(
                        yT[:, b, LQ[b]:T],
                        ypad[:, b:b + 1].to_broadcast([P, T - LQ[b]]))

            obtiles = {}

            def out_chunk(b, mt, qb):
                if qb == 0:
                    obtiles[b, mt] = obpool.tile([P, T], BF16, tag="ob",
                                                 name=f"ob{b}_{mt}")
                ob = obtiles[b, mt]
                ps = psS.tile([P, 512], F32, tag="psS", name=f"o{b}_{mt}_{qb}")
                nc.tensor.matmul(ps, wp_sb[:, mt * P:(mt + 1) * P],
                                 yT[:, b, qb * 512:(qb + 1) * 512],
                                 start=True, stop=True)
                if (mt + qb) % 4 == 0:
                    nc.vector.tensor_copy(ob[:, qb * 512:(qb + 1) * 512], ps)
                else:
                    nc.scalar.copy(ob[:, qb * 512:(qb + 1) * 512], ps)
                if qb == 3:
                    nc.gpsimd.dma_start(oT[b, mt * P:(mt + 1) * P, :], ob)

            def out_ops(b):
                return [lambda b=b, mt=mt, qb=qb: out_chunk(b, mt, qb)
                        for mt in range(8) for qb in range(4)]

            # ---- schedule (smallest batch first; the biggest batch runs
            # last so its attention hides the previous batch's epilogue) ----
            order = sorted(range(B), key=lambda b: kb[b])
            b0 = order[0]
            load_x(b0)
            pre = prep_batch(b0)
            for op in pre[:3]:
                op()
            bg0 = pre[3:]
            for i, b in enumerate(order):
                bg = bg0 if i == 0 else out_ops(order[i - 1])
                if i + 1 < B:
                    load_x(order[i + 1])
                    bg = bg + prep_batch(order[i + 1])
                attention(b, bg, need=(lambda j: 3 * j) if i == 0 else None)
                norm_batch(b)
            for op in out_ops(order[-1]):
                op()

            if debug:
                for b in range(B):
                    nc.gpsimd.dma_start(d_q[:, b, 0:LQ[b]], qT[:, b, 0:LQ[b]])
                    nc.gpsimd.dma_start(d_k[:, b, 0:LQ[b]], kT[:, b, 0:LQ[b]])
                nc.gpsimd.dma_start(d_V[:], V_sb)
                nc.gpsimd.dma_start(d_y[:], yT)
                nc.gpsimd.dma_start(d_yp[:], ypad)

    nc.compile()
    return nc


def _bf(a):
    return np.ascontiguousarray(np.asarray(a)).astype(ml_dtypes.bfloat16)


def _prep_inputs(x, l, W_attn, W_proj):
    x = np.asarray(x, dtype=np.float32)
    W_attn = np.asarray(W_attn, dtype=np.float32)
    W_proj = np.asarray(W_proj, dtype=np.float32)
    lv = np.clip(np.asarray(l).astype(np.int64), 1, T)
    kbs = [int(math.ceil(int(lv[b]) / 128.0)) for b in range(B)]

    xTb = np.stack([np.ascontiguousarray(x[b].T) for b in range(B)])
    xTb = xTb.astype(ml_dtypes.bfloat16)

    m01 = np.where(np.arange(P)[:, None] > np.arange(P)[None, :],
                   0.0, 1.0).astype(ml_dtypes.bfloat16)

    qmA = np.zeros((P, 512), dtype=np.float32)
    qmnt = np.zeros((P, B, 512), dtype=np.float32)
    for b in range(B):
        lb = int(lv[b])
        jb = (kbs[b] + 3) // 4
        qrow = (np.arange(T) < lb).astype(np.float32)
        for j in range(jb):
            seg = qrow[512 * j:512 * (j + 1)]
            for par in range(2):
                qmA[b * 32 + par * 4 + j, :] = seg
        tail = qrow[512 * (jb - 1):512 * jb]
        qmnt[:, b, 0:len(tail)] = 1.0 - tail[None, :]

    in_maps = []
    for c in range(8):
        cs = slice(c * P, (c + 1) * P)
        wqc = _bf(W_attn[:, 0 * C:1 * C][:, cs].reshape(8, P, P).transpose(1, 0, 2))
        wkc = _bf(W_attn[:, 1 * C:2 * C][:, cs].reshape(8, P, P).transpose(1, 0, 2))
        wvc = _bf(W_attn[:, 2 * C:3 * C][:, cs].reshape(8, P, P).transpose(1, 0, 2))
        wpc = _bf(W_proj[cs, :])
        in_maps.append({
            "xT": xTb, "wq": wqc, "wk": wkc, "wv": wvc, "wp": wpc,
            "ident": np.tile(np.eye(D, dtype=np.float32),
                             (2, 1)).astype(ml_dtypes.bfloat16),
            "m01": m01, "qmA": qmA, "qmnt": qmnt,
        })
    return in_maps, tuple(kbs)


def kernel(x, l, W_attn, b_attn, W_proj, b_proj, _want_profile=False):
    b_attn = np.asarray(b_attn, dtype=np.float32)
    b_proj = np.asarray(b_proj, dtype=np.float32)
    assert not np.any(b_attn), "nonzero b_attn not supported by this kernel"

    in_maps, kbs = _prep_inputs(x, l, W_attn, W_proj)
    if kbs not in _CACHED:
        _CACHED[kbs] = build_nc(kbs)
    nc = _CACHED[kbs]

    res = run_bass_kernel_spmd(nc, in_maps, core_ids=list(range(8)),
                               trace=_want_profile)

    acc = np.zeros((B, C, T), dtype=np.float32)
    for c in range(8):
        acc += res.results[c]["oT"].astype(np.float32)
    out = np.ascontiguousarray(acc.transpose(0, 2, 1)) + b_proj[None, None, :]
    if _want_profile:
        return out, res
    return out


# revision 59
# speedup vs baseline: 1.1194x; 1.0436x over previous
"""Trainium2 Bass kernel for nn_CausalSelfAttention_17368847745133.

Sharding (8 NeuronCores): core c owns heads (2c, 2c+1) for ALL 4 batches
(Megatron column-parallel c_attn / row-parallel c_proj over heads).  The
program is specialized at build time on Kb[b] = ceil(l_b/128): query/key
projections, S = qk^T, exp, and PV are emitted only for the live region
q,k < 128*Kb[b]; rows q >= l_b of the output are the uniform-softmax mean
of v (the reference's -1e8 mask makes those rows exactly uniform), blended
in with host-provided 0/1 masks, so results stay correct for the given l.

Per core / batch b (all matmuls bf16, statistics fp32):
  V^T  [128,2048] : matmul(lhsT=Wv slice, rhs=xT)    then one
                    dma_start_transpose per head -> V [kpos,2,65] tiles
                    (col 64 = ones for the softmax denominator row)
  ypad [128,1]    : DVE row-sum of V^T / T   (mean of v over ALL k)
  qT/kT[128,LQ]   : matmul(lhsT=Wq|k slice, rhs=xT), trimmed to LQ
  S^T  [k,q]      : matmul(lhsT=kT head, rhs=qT head), head pair packed
                    on PE row groups 0-63 / 64-127, block-causal trimmed
  P^T  = exp(S^T/8) * m01 on diagonal blocks
  U    [65,q]     : matmul(lhsT=[V|1], rhs=P^T); row 64 = denominator
  y^T  = U[0:64]*bcast(qm/den) + ypad*(1-qm);  rows q >= 128*Kb = ypad
  o^T  partial [1024,2048] = matmul(lhsT=Wp rows, rhs=y^T); host sums the
                    8 per-core partials, transposes, adds b_proj.

Batch b+1's V^T and q/k projections are interleaved into batch b's
attention j-loop to keep the PE busy while the ACT engine runs exp.
"""

import math

import ml_dtypes
import numpy as np

import concourse.bass as bass
import concourse.mybir as mybir
import concourse.tile as tile
from concourse import bacc
from concourse.bass_utils import run_bass_kernel_spmd

P = 128
B, T, C = 4, 2048, 1024
H, D = 16, 64
F32 = mybir.dt.float32
BF16 = mybir.dt.bfloat16
SCALE = 0.125    # 1/sqrt(64)

_CACHED = {}
_POPS = True


def build_nc(kb, debug=False):
    """kb: tuple of 4 ints, Kb[b] = ceil(clamp(l_b,1,T)/128) in 1..16."""
    nc = bacc.Bacc(trn_type="TRN2", target_bir_lowering=False)

    xT = nc.dram_tensor("xT", [B, C, T], BF16, kind="ExternalInput")
    wq = nc.dram_tensor("wq", [P, 8, P], BF16, kind="ExternalInput")
    wk = nc.dram_tensor("wk", [P, 8, P], BF16, kind="ExternalInput")
    wv = nc.dram_tensor("wv", [P, 8, P], BF16, kind="ExternalInput")
    wp = nc.dram_tensor("wp", [P, C], BF16, kind="ExternalInput")
    ident = nc.dram_tensor("ident", [P, D], BF16, kind="ExternalInput")
    m01 = nc.dram_tensor("m01", [P, P], BF16, kind="ExternalInput")
    qmA = nc.dram_tensor("qmA", [P, 512], F32, kind="ExternalInput")
    qmnt = nc.dram_tensor("qmnt", [P, B, 512], F32, kind="ExternalInput")
    oT = nc.dram_tensor("oT", [B, C, T], BF16, kind="ExternalOutput")
    if debug:
        d_q = nc.dram_tensor("d_q", [P, B, T], BF16, kind="ExternalOutput")
        d_k = nc.dram_tensor("d_k", [P, B, T], BF16, kind="ExternalOutput")
        d_V = nc.dram_tensor("d_V", [P, B, 16, 2, D + 1], BF16,
                             kind="ExternalOutput")
        d_y = nc.dram_tensor("d_y", [P, B, T], BF16, kind="ExternalOutput")
        d_yp = nc.dram_tensor("d_yp", [P, B], F32, kind="ExternalOutput")

    LQ = [128 * k for k in kb]
    JB = [(k + 3) // 4 for k in kb]
    QE = [LQ[b] - 512 * (JB[b] - 1) for b in range(B)]

    with tile.TileContext(nc) as tc:
        with tc.tile_pool(name="misc", bufs=1) as misc, \
             tc.tile_pool(name="xp", bufs=2) as xpool, \
             tc.tile_pool(name="vtp", bufs=2) as vtpool, \
             tc.tile_pool(name="qk", bufs=1) as qkpool, \
             tc.tile_pool(name="vp", bufs=1) as vpool, \
             tc.tile_pool(name="yp", bufs=1) as ypool, \
             tc.tile_pool(name="pt", bufs=4) as ptpool, \
             tc.tile_pool(name="nrm", bufs=3) as nrm, \
             tc.tile_pool(name="ob", bufs=3) as obpool, \
             tc.tile_pool(name="rdram", bufs=2, space="DRAM") as rdram, \
             tc.tile_pool(name="psS", bufs=3, space="PSUM") as psS, \
             tc.tile_pool(name="psU", bufs=2, space="PSUM") as psU:

            # ---- constants ----
            m01_sb = misc.tile([P, P], BF16, tag="m01")
            qmA_sb = misc.tile([P, 512], F32, tag="qmA")
            qmnt_sb = misc.tile([P, B, 512], F32, tag="qmnt")
            wq_sb = misc.tile([P, 8, P], BF16, tag="wq")
            wk_sb = misc.tile([P, 8, P], BF16, tag="wk")
            wv_sb = misc.tile([P, 8, P], BF16, tag="wv")
            wp_sb = misc.tile([P, C], BF16, tag="wp")
            id_sb = misc.tile([P, D], BF16, tag="ident")
            for dst, src in ((m01_sb, m01), (qmA_sb, qmA), (qmnt_sb, qmnt),
                             (wq_sb, wq), (wk_sb, wk), (wv_sb, wv),
                             (wp_sb, wp), (id_sb, ident)):
                nc.sync.dma_start(dst, src[:])

            qT = qkpool.tile([P, B, T], BF16, tag="qT")
            kT = qkpool.tile([P, B, T], BF16, tag="kT")
            V_sb = vpool.tile([P, B, 16, 2, D + 1], BF16, tag="V")
            yT = ypool.tile([P, B, T], BF16, tag="yT")
            ypad = misc.tile([P, B], F32, tag="ypad")
            den_sb = misc.tile([P, 512], F32, tag="den")
            nc.vector.memset(den_sb, 1.0)
            for b in range(B):
                nc.vector.memset(V_sb[:, b, :, :, D:D + 1], 1.0)

            xtiles = {}

            def load_x(b):
                xb = xpool.tile([P, 8, T], BF16, tag="x", name=f"x{b}")
                for ct in range(8):
                    nc.sync.dma_start(xb[:, ct], xT[b, ct * P:(ct + 1) * P, :])
                xtiles[b] = xb

            def v_proj_chunk(b, pb):
                """V^T projection for 512 positions (pb in 0..3), transposed
                into V_sb k-tiles 4*pb..4*pb+3 via PE transpose
                (dma_start_transpose is not dependency-tracked by Tile and
                races its consumers on HW)."""
                xb = xtiles[b]
                vtp = self_vtp[b]
                ps = psS.tile([P, 512], F32, tag="psS", name=f"vp{b}_{pb}")
                for kt in range(8):
                    nc.tensor.matmul(ps, wv_sb[:, kt, :],
                                     xb[:, kt, pb * 512:(pb + 1) * 512],
                                     start=(kt == 0), stop=(kt == 7))
                nc.vector.tensor_copy(vtp[:, pb * 512:(pb + 1) * 512], ps)
                for h in range(2):
                    psv = psS.tile([P, 4, D], BF16, tag="psS",
                                   name=f"tp{b}_{pb}_{h}")
                    for t in range(4):
                        nc.tensor.transpose(
                            psv[:, t, :],
                            vtp[h * D:(h + 1) * D,
                                (4 * pb + t) * P:(4 * pb + t + 1) * P],
                            id_sb[h * D:(h + 1) * D, :])
                    nc.vector.tensor_copy(
                        V_sb[:, b, 4 * pb:4 * pb + 4, h, 0:D], psv)

            def ypad_reduce(b):
                vtp = self_vtp[b]
                nc.vector.tensor_reduce(
                    ypad[:, b:b + 1], vtp, axis=mybir.AxisListType.X,
                    op=mybir.AluOpType.add)
                nc.vector.tensor_scalar_mul(ypad[:, b:b + 1],
                                            ypad[:, b:b + 1], 1.0 / T)

            def qk_chunk(b, side, ch):
                """q or k projection for 512 positions; ch in 0..ceil(LQ/512)-1."""
                xb = xtiles[b]
                w_sb = (wq_sb, wk_sb)[side]
                dst = (qT, kT)[side]
                width = min(512, LQ[b] - 512 * ch)
                ps = psS.tile([P, 512], F32, tag="psS", name=f"qk{b}_{side}_{ch}")
                for kt in range(8):
                    nc.tensor.matmul(ps[:, 0:width], w_sb[:, kt, :],
                                     xb[:, kt, 512 * ch:512 * ch + width],
                                     start=(kt == 0), stop=(kt == 7))
                nc.vector.tensor_copy(dst[:, b, 512 * ch:512 * ch + width],
                                      ps[:, 0:width])

            self_vtp = {}

            def prep_batch(b):
                """Emit all projection work for batch b as a flat op list,
                ordered so the earliest attention blocks' needs come first."""
                self_vtp[b] = vtpool.tile([P, T], BF16, tag="vt", name=f"vt{b}")
                nch = (LQ[b] + 511) // 512
                ops = []
                for i in range(4):
                    ops.append(lambda b=b, pb=i: v_proj_chunk(b, pb))
                    if i < nch:
                        ops.append(lambda b=b, ch=i: qk_chunk(b, 0, ch))
                        ops.append(lambda b=b, ch=i: qk_chunk(b, 1, ch))
                ops.append(lambda b=b: ypad_reduce(b))
                return ops

            def attention(b, bg_ops, need=None):
                """Attention for batch b; bg_ops are interleaved (next batch
                projections) to fill PE time while ACT runs exp.  need(j)
                gives the minimum number of bg_ops that must be emitted
                before block j (self-projection ordering for the first
                batch)."""
                Kb = kb[b]
                popped = [0]

                def pop():
                    bg_ops.pop(0)()
                    popped[0] += 1

                for j in range(JB[b]):
                    if need is not None:
                        while bg_ops and popped[0] < need(j):
                            pop()
                    Nj = 512 if j < JB[b] - 1 else QE[b]
                    nkt = min(4 * (j + 1), Kb)
                    blk = slice(512 * j, 512 * j + Nj)
                    Upr = [psU.tile([D + 1, 512], F32, tag="psU",
                                    name=f"U{b}_{j}_{par}") for par in range(2)]

                    def s_exp(kt, j=j, Nj=Nj):
                        dlt = 128 * kt - 512 * j
                        c0 = max(dlt, 0)
                        ss = psS.tile([P, 2, 512], F32, tag="psS")
                        for par in range(2):
                            p0 = par * D
                            nc.tensor.matmul(
                                ss[:, par, c0:Nj],
                                kT[p0:p0 + D, b, kt * P:(kt + 1) * P],
                                qT[p0:p0 + D, b, 512 * j + c0:512 * j + Nj],
                                start=True, stop=True)
                        pt = ptpool.tile([P, 2, 512], BF16, tag="pt")
                        nc.scalar.activation(
                            pt[:, :, c0:Nj], ss[:, :, c0:Nj],
                            mybir.ActivationFunctionType.Exp,
                            bias=0.0, scale=SCALE)
                        ptm = None
                        if dlt >= 0:
                            # masked diagonal tile goes to its own tile so the
                            # PV matmul has a hard dependency on the mask
                            ptm = ptpool.tile([P, 2, P], BF16, tag="ptm")
                            nc.vector.tensor_mul(
                                out=ptm,
                                in0=pt[:, :, c0:c0 + P],
                                in1=m01_sb[:, None, :].to_broadcast([P, 2, P]))
                        return pt, ptm, c0

                    def pv(kt, pt, ptm, c0, Nj=Nj, nkt=nkt):
                        # columns < c0 are causally dead for this k tile:
                        # accumulate only the live range (kt==0 has c0==0,
                        # so every column is initialized by the first step)
                        for par in range(2):
                            if ptm is None:
                                nc.tensor.matmul(
                                    Upr[par][:, c0:Nj],
                                    V_sb[:, b, kt, par, :],
                                    pt[:, par, c0:Nj],
                                    start=(kt == 0), stop=(kt == nkt - 1),
                                    skip_group_check=True)
                            else:
                                nc.tensor.matmul(
                                    Upr[par][:, c0:c0 + P],
                                    V_sb[:, b, kt, par, :],
                                    ptm[:, par, :],
                                    start=(kt == 0),
                                    stop=(kt == nkt - 1 and c0 + P >= Nj),
                                    skip_group_check=True)
                                if c0 + P < Nj:
                                    nc.tensor.matmul(
                                        Upr[par][:, c0 + P:Nj],
                                        V_sb[:, b, kt, par, :],
                                        pt[:, par, c0 + P:Nj],
                                        start=(kt == 0), stop=(kt == nkt - 1),
                                        skip_group_check=True)

                    prev = None
                    for kt in range(nkt):
                        cur = s_exp(kt)
                        if prev is not None:
                            pv(kt - 1, *prev)
                        prev = cur
                        if bg_ops and j > 0 and _POPS:
                            pop()
                    pv(nkt - 1, *prev)

                    # per-j epilogue: stash denom row + unnormalized y
                    for par in range(2):
                        U = Upr[par]
                        r = b * 32 + par * 4 + j
                        dtf = nrm.tile([D + 1, 512], F32, tag="dt")
                        nc.vector.tensor_copy(dtf[D:D + 1, 0:Nj],
                                              U[D:D + 1, 0:Nj])
                        nc.gpsimd.dma_start(den_sb[r:r + 1, 0:Nj],
                                            dtf[D:D + 1, 0:Nj])
                        if par == 0:
                            nc.vector.tensor_copy(yT[0:D, b, blk], U[0:D, 0:Nj])
                        else:
                            yt2 = nrm.tile([D, 512], BF16, tag="ytmp")
                            nc.vector.tensor_copy(yt2[:, 0:Nj], U[0:D, 0:Nj])
                            nc.gpsimd.dma_start(yT[D:P, b, blk], yt2[:, 0:Nj])
                while bg_ops:
                    bg_ops.pop(0)()

            def norm_batch(b):
                r0 = b * 32
                dq = den_sb[r0:r0 + 8, :]
                nc.vector.reciprocal(dq, dq)
                nc.vector.tensor_mul(out=dq, in0=dq, in1=qmA_sb[r0:r0 + 8, :])
                dend = rdram.tile([8, 512], F32, tag="dend", name=f"dend{b}")
                nc.sync.dma_start(dend, dq)
                for j in range(JB[b]):
                    Nj = 512 if j < JB[b] - 1 else QE[b]
                    blk = slice(512 * j, 512 * j + Nj)
                    rb = nrm.tile([P, 512], F32, tag="rb")
                    for par in range(2):
                        row = dend[par * 4 + j:par * 4 + j + 1, 0:Nj]
                        src = bass.AP(tensor=row.tensor, offset=row.offset,
                                      ap=[[0, D]] + list(row.ap[1:]))
                        nc.sync.dma_start(rb[par * D:(par + 1) * D, 0:Nj], src)
                    ys = yT[:, b, blk]
                    nc.vector.tensor_mul(out=ys, in0=ys, in1=rb[:, 0:Nj])
                    if j == JB[b] - 1:
                        t3 = nrm.tile([P, 512], F32, tag="t3")
                        nc.vector.tensor_mul(
                            out=t3[:, 0:Nj], in0=qmnt_sb[:, b, 0:Nj],
                            in1=ypad[:, b:b + 1].to_broadcast([P, Nj]))
                        nc.vector.tensor_add(out=ys, in0=ys, in1=t3[:, 0:Nj])
                if LQ[b] < T:
                    nc.vector.tensor_copy(
                        yT[:, b, LQ[b]:T],
                        ypad[:, b:b + 1].to_broadcast([P, T - LQ[b]]))

            obtiles = {}

            def out_chunk(b, mt, qb):
                if qb == 0:
                    obtiles[b, mt] = obpool.tile([P, T], BF16, tag="ob",
                                                 name=f"ob{b}_{mt}")
                ob = obtiles[b, mt]
                ps = psS.tile([P, 512], F32, tag="psS", name=f"o{b}_{mt}_{qb}")
                nc.tensor.matmul(ps, wp_sb[:, mt * P:(mt + 1) * P],
                                 yT[:, b, qb * 512:(qb + 1) * 512],
                                 start=True, stop=True)
                if (mt + qb) % 4 != 3:
                    nc.vector.tensor_copy(ob[:, qb * 512:(qb + 1) * 512], ps)
                else:
                    nc.scalar.copy(ob[:, qb * 512:(qb + 1) * 512], ps)
                if qb == 3:
                    nc.gpsimd.dma_start(oT[b, mt * P:(mt + 1) * P, :], ob)

            def out_ops(b):
                return [lambda b=b, mt=mt, qb=qb: out_chunk(b, mt, qb)
                        for mt in range(8) for qb in range(4)]

            # ---- schedule (smallest batch first; the biggest batch runs
            # last so its attention hides the previous batch's epilogue) ----
            order = sorted(range(B), key=lambda b: kb[b])
            b0 = order[0]
            load_x(b0)
            pre = prep_batch(b0)
            for op in pre[:3]:
                op()
            bg0 = pre[3:]
            for i, b in enumerate(order):
                bg = bg0 if i == 0 else out_ops(order[i - 1])
                if i + 1 < B:
                    load_x(order[i + 1])
                    bg = bg + prep_batch(order[i + 1])
                attention(b, bg, need=(lambda j: 3 * j) if i == 0 else None)
                norm_batch(b)
            for op in out_ops(order[-1]):
                op()

            if debug:
                for b in range(B):
                    nc.gpsimd.dma_start(d_q[:, b, 0:LQ[b]], qT[:, b, 0:LQ[b]])
                    nc.gpsimd.dma_start(d_k[:, b, 0:LQ[b]], kT[:, b, 0:LQ[b]])
                nc.gpsimd.dma_start(d_V[:], V_sb)
                nc.gpsimd.dma_start(d_y[:], yT)
                nc.gpsimd.dma_start(d_yp[:], ypad)

    nc.compile()
    return nc


def _bf(a):
    return np.ascontiguousarray(np.asarray(a)).astype(ml_dtypes.bfloat16)


def _prep_inputs(x, l, W_attn, W_proj):
    x = np.asarray(x, dtype=np.float32)
    W_attn = np.asarray(W_attn, dtype=np.float32)
    W_proj = np.asarray(W_proj, dtype=np.float32)
    lv = np.clip(np.asarray(l).astype(np.int64), 1, T)
    kbs = [int(math.ceil(int(lv[b]) / 128.0)) for b in range(B)]

    xTb = np.stack([np.ascontiguousarray(x[b].T) for b in range(B)])
    xTb = xTb.astype(ml_dtypes.bfloat16)

    m01 = np.where(np.arange(P)[:, None] > np.arange(P)[None, :],
                   0.0, 1.0).astype(ml_dtypes.bfloat16)

    qmA = np.zeros((P, 512), dtype=np.float32)
    qmnt = np.zeros((P, B, 512), dtype=np.float32)
    for b in range(B):
        lb = int(lv[b])
        jb = (kbs[b] + 3) // 4
        qrow = (np.arange(T) < lb).astype(np.float32)
        for j in range(jb):
            seg = qrow[512 * j:512 * (j + 1)]
            for par in range(2):
                qmA[b * 32 + par * 4 + j, :] = seg
        tail = qrow[512 * (jb - 1):512 * jb]
        qmnt[:, b, 0:len(tail)] = 1.0 - tail[None, :]

    in_maps = []
    for c in range(8):
        cs = slice(c * P, (c + 1) * P)
        wqc = _bf(W_attn[:, 0 * C:1 * C][:, cs].reshape(8, P, P).transpose(1, 0, 2))
        wkc = _bf(W_attn[:, 1 * C:2 * C][:, cs].reshape(8, P, P).transpose(1, 0, 2))
        wvc = _bf(W_attn[:, 2 * C:3 * C][:, cs].reshape(8, P, P).transpose(1, 0, 2))
        wpc = _bf(W_proj[cs, :])
        in_maps.append({
            "xT": xTb, "wq": wqc, "wk": wkc, "wv": wvc, "wp": wpc,
            "ident": np.tile(np.eye(D, dtype=np.float32),
                             (2, 1)).astype(ml_dtypes.bfloat16),
            "m01": m01, "qmA": qmA, "qmnt": qmnt,
        })
    return in_maps, tuple(kbs)


def kernel(x, l, W_attn, b_attn, W_proj, b_proj, _want_profile=False):
    b_attn = np.asarray(b_attn, dtype=np.float32)
    b_proj = np.asarray(b_proj, dtype=np.float32)
    assert not np.any(b_attn), "nonzero b_attn not supported by this kernel"

    in_maps, kbs = _prep_inputs(x, l, W_attn, W_proj)
    if kbs not in _CACHED:
        _CACHED[kbs] = build_nc(kbs)
    nc = _CACHED[kbs]

    res = run_bass_kernel_spmd(nc, in_maps, core_ids=list(range(8)),
                               trace=_want_profile)

    acc = np.zeros((B, C, T), dtype=np.float32)
    for c in range(8):
        acc += res.results[c]["oT"].astype(np.float32)
    out = np.ascontiguousarray(acc.transpose(0, 2, 1)) + b_proj[None, None, :]
    if _want_profile:
        return out, res
    return out
